# revision 1
# baseline (speedup 1.0000x reference)
"""GATv2Conv-with-edge-features Trainium2 kernel (8-core SPMD, edge-sharded by dst).

Self-contained: hardcodes problem shapes (N=50000 nodes, E=800000 edges,
128 feat, 8 heads x 16). Core k owns dst nodes [6250k, 6250(k+1)) and the
edges pointing into them. Edges are sorted by dst and packed into tiles of
<=128 edges covering <=32 consecutive dst nodes; tile windows PARTITION the
local node range so every node (incl. degree-0) has exactly one slot.

Per 16-tile super-tile (2048 edges):
  - ONE interleaved mega-DMA brings x16[src], x8[dst], ef16 (feature-major
    per tile) and the one-hot scatter matrices S16,
  - T = xs@Ws + xd@Wd + ef@We: fp16 matmuls for the src/edge terms and an
    fp8 DoubleRow matmul for the dst term (stride-0 dual slab against
    hi/lo-split fp8 W_dst); the dst term's fp8 error cancels exactly in
    phase C, which recomputes fdst with the identical matmul,
  - score = attn . leaky(T) via leaky(x) = 0.6x + 0.4|x|: the linear part
    comes from tiny matmuls against device-built w~ = W^T (0.6 attn), |T|
    is one scalar-engine Abs pass, 0.4*attn is applied on DVE (fp16 2x),
    the d-reduction runs as a pairwise tree (Pool steps 1-3, DVE step 4),
    and the result is folded into the lin PSUM by an identity matmul so
    exp reads a single PSUM operand,
  - ex = exp(score - 4) written by the scalar engine as fp16 directly into
    the z-columns of the msg buffer; msg = T * ex on DVE (PSUM read),
  - scatter into 3-tile column groups at PSUM partition bases {0,32,64}
    (one plain fp16 matmul per tile; z rides as 8 extra columns),
  - U|z evacuated fp16 by one scalar-engine pass per half, one DMA per ST.
Phase C: dma_gather pulls 512 U-rows (512B each) per Pool instruction and
out = relu((U - fdst*z) * (1/max(z,eps))) needs no z=0 mask.
"""
import numpy as np
import ml_dtypes

import concourse.bacc as bacc
import concourse.tile as tile
import concourse.mybir as mybir
from concourse.bass_utils import run_bass_kernel_spmd

N_NODES = 50000
N_CORES = 8
N_LOCAL = N_NODES // N_CORES          # 6250
IN_FEAT = 128
HEADS = 8
HEAD_DIM = 16
TILE_E = 128
TILE_W = 32
ST_TILES = 16
EXP_SHIFT = 4.0
P = 128
FP = mybir.dt.float32
F16 = mybir.dt.float16
F8 = mybir.dt.float8e4
U8 = mybir.dt.uint8
I16 = mybir.dt.int16
NP8 = ml_dtypes.float8_e4m3
NP16 = np.float16

TILE_BYTES = 640                      # xs16 256 | xd8 128 | ef16 256
S_BYTES = 64                          # S16 per tile (9th block = zeros)
HALF_BYTES = 8 * TILE_BYTES + 9 * S_BYTES    # 5696
ST_BYTES = 2 * HALF_BYTES                    # 11392
ST_SLOTS = 576                        # 2 halves x 3 groups x 96 slots
ROW_E = 256                           # U_d row: 128 feat + 8 z + 120 pad (fp16)
BATCH_N = 512
DR = mybir.MatmulPerfMode.DoubleRow
N_CHUNKS = (N_LOCAL + P - 1) // P     # 49
NB = (N_LOCAL + BATCH_N - 1) // BATCH_N  # 13


# ---------------------------------------------------------------- host prep

def _pack_core(dst_local, n_local):
    """Tiles of <=TILE_E edges whose dst windows PARTITION [0, n_local)."""
    deg = np.bincount(dst_local, minlength=n_local)
    tb = [0]
    cnt = 0
    for n in range(n_local):
        d = int(deg[n])
        assert d <= TILE_E, f"node degree {d} exceeds tile capacity {TILE_E}"
        if (cnt + d > TILE_E or n - tb[-1] >= TILE_W) and n > tb[-1]:
            tb.append(n)
            cnt = 0
        cnt += d
    tb = np.asarray(tb + [n_local])
    t_of_node = np.searchsorted(tb, np.arange(n_local), side="right") - 1
    s_of_node = np.arange(n_local) - tb[t_of_node]
    tile_cnt = np.array([int(deg[tb[t]:tb[t + 1]].sum())
                         for t in range(len(tb) - 1)])
    assert (tile_cnt <= TILE_E).all()
    return tb, tile_cnt, t_of_node, s_of_node


def _slot_of(t, s):
    """Global U_d row for (tile, within-window offset)."""
    tl = t % ST_TILES
    return ((t // ST_TILES) * ST_SLOTS + (tl // 8) * 288
            + ((tl % 8) // 3) * 96 + ((tl % 8) % 3) * 32 + s)


def _prep_cores(x, efeat, src, dst, W_src, b_src, W_dst, b_dst, W_edge, attn):
    x = np.ascontiguousarray(np.asarray(x, np.float32))
    efeat = np.asarray(efeat, np.float32)
    src = np.asarray(src).astype(np.int64)
    dst = np.asarray(dst).astype(np.int64)
    W_src = np.asarray(W_src, np.float32)
    W_dst = np.asarray(W_dst, np.float32)
    W_edge = np.asarray(W_edge, np.float32)
    attn = np.asarray(attn, np.float32)
    assert np.abs(np.asarray(b_src)).max() == 0
    assert np.abs(np.asarray(b_dst)).max() == 0

    x16 = x.astype(NP16)
    x8 = x.astype(NP8)
    ef16 = efeat.astype(NP16)

    per_core = []
    core_T = []
    for k in range(N_CORES):
        lo = k * N_LOCAL
        eidx = np.nonzero((dst >= lo) & (dst < lo + N_LOCAL))[0]
        dl = (dst[eidx] - lo).astype(np.int64)
        order = np.argsort(dl, kind="stable")
        eidx, dl = eidx[order], dl[order]
        per_core.append((eidx, dl) + _pack_core(dl, N_LOCAL))
        core_T.append(len(per_core[-1][2]) - 1)

    T_tiles = max(core_T)
    T_tiles = ((T_tiles + ST_TILES - 1) // ST_TILES) * ST_TILES

    Wd_hi = np.ascontiguousarray(W_dst.T).astype(NP8)
    Wd_lo = (W_dst.T - Wd_hi.astype(np.float32)).astype(NP8)
    Wdp = np.ascontiguousarray(np.concatenate([Wd_hi, Wd_lo], axis=1))
    WsT16 = np.ascontiguousarray(W_src.T.astype(NP16))
    WeT16 = np.ascontiguousarray(W_edge.T.astype(NP16))
    # attn split: 0.4 on the |T| path (attn_rep), 0.6 on the linear path
    attn_diag = np.zeros((P, HEADS), NP16)
    for h in range(HEADS):
        attn_diag[h * HEAD_DIM:(h + 1) * HEAD_DIM, h] = \
            (0.6 * attn[h]).astype(NP16)
    attn_rep = np.ascontiguousarray(
        np.broadcast_to((0.4 * attn).reshape(1, HEADS * HEAD_DIM),
                        (P, HEADS * HEAD_DIM)).astype(NP16))
    ident16 = np.eye(P, dtype=NP16)

    in_maps = []
    for k in range(N_CORES):
        eidx, dl, tb, tcnt, t_of_node, s_of_node = per_core[k]
        nt = len(tb) - 1

        mega = np.zeros((P, (T_tiles // ST_TILES) * ST_BYTES), np.uint8)
        pos = 0
        for t in range(nt):
            c = int(tcnt[t])
            if c == 0:
                continue
            st, tl8 = t // ST_TILES, (t % ST_TILES) % 8
            half = (t % ST_TILES) // 8
            base = st * ST_BYTES + half * HALF_BYTES + tl8 * TILE_BYTES
            e_ids = eidx[pos:pos + c]
            d_loc = dl[pos:pos + c]
            pos += c
            mega[:, base:base + 2 * c].view(NP16)[:, :c] = x16[src[e_ids]].T
            mega[:, base + 256:base + 256 + c].view(NP8)[:, :c] = \
                x8[d_loc + k * N_LOCAL].T
            mega[:, base + 384:base + 384 + 2 * c].view(NP16)[:, :c] = \
                ef16[e_ids].T
            sbase = st * ST_BYTES + half * HALF_BYTES + 8 * TILE_BYTES \
                + tl8 * S_BYTES
            sl = d_loc - tb[t]
            sview = mega[:, sbase:sbase + S_BYTES].view(NP16)
            sview[np.arange(c), sl] = NP16(1.0)
        assert pos == len(eidx)

        xTl8 = np.zeros((P, N_CHUNKS * P), NP8)
        xTl8[:, :N_LOCAL] = x8[k * N_LOCAL:(k + 1) * N_LOCAL].T

        node_slot = _slot_of(t_of_node, s_of_node)
        sm = np.zeros(NB * BATCH_N, np.int16)
        sm[:N_LOCAL] = node_slot.astype(np.int16)
        smap16 = np.ascontiguousarray(np.tile(
            sm.reshape(NB, BATCH_N // 16, 16).transpose(2, 0, 1)
            .reshape(16, -1), (8, 1)))                   # [128, NB*32]

        in_maps.append(dict(
            mega_in=mega,
            Wdp=Wdp, WsT16=WsT16, WeT16=WeT16,
            Ws16=W_src.astype(NP16), Wd16=W_dst.astype(NP16),
            We16=W_edge.astype(NP16),
            attn_diag=attn_diag, attn_rep=attn_rep, ident16=ident16,
            xTl8=xTl8, smap16=smap16,
        ))
    return in_maps, T_tiles


# ------------------------------------------------------------- bass program

def build_program(T_tiles):
    nc = bacc.Bacc("TRN2", target_bir_lowering=False, debug=False,
                   num_devices=N_CORES)
    n_st = T_tiles // ST_TILES

    mega_d = nc.dram_tensor("mega_in", [P, n_st * ST_BYTES], U8,
                            kind="ExternalInput")
    Wdp_d = nc.dram_tensor("Wdp", [P, 2 * IN_FEAT], F8, kind="ExternalInput")
    WsT_d = nc.dram_tensor("WsT16", [P, IN_FEAT], F16, kind="ExternalInput")
    WeT_d = nc.dram_tensor("WeT16", [P, IN_FEAT], F16, kind="ExternalInput")
    Ws16_d = nc.dram_tensor("Ws16", [P, IN_FEAT], F16, kind="ExternalInput")
    Wd16_d = nc.dram_tensor("Wd16", [P, IN_FEAT], F16, kind="ExternalInput")
    We16_d = nc.dram_tensor("We16", [P, IN_FEAT], F16, kind="ExternalInput")
    adiag_d = nc.dram_tensor("attn_diag", [P, HEADS], F16, kind="ExternalInput")
    arep_d = nc.dram_tensor("attn_rep", [P, IN_FEAT], F16, kind="ExternalInput")
    ident_d = nc.dram_tensor("ident16", [P, P], F16, kind="ExternalInput")
    xTl8_d = nc.dram_tensor("xTl8", [P, N_CHUNKS * P], F8, kind="ExternalInput")
    smap_d = nc.dram_tensor("smap16", [P, NB * (BATCH_N // 16)], I16,
                            kind="ExternalInput")

    U_d = nc.dram_tensor("U_i", [n_st * ST_SLOTS, ROW_E], F16, kind="Internal")
    out_d = nc.dram_tensor("out", [N_LOCAL, IN_FEAT], FP, kind="ExternalOutput")

    HALF_E = 8 * TILE_E                  # 1024

    with tile.TileContext(nc) as tc:
        with tc.tile_pool(name="const", bufs=1) as cb:
            def cload(name, shape, dt, dram):
                t = cb.tile(shape, dt, name=name)
                nc.sync.dma_start(out=t[:], in_=dram[:])
                return t

            Wdp = cload("Wdp_s", [P, 2 * IN_FEAT], F8, Wdp_d)
            WsT = cload("WsT_s", [P, IN_FEAT], F16, WsT_d)
            WeT = cload("WeT_s", [P, IN_FEAT], F16, WeT_d)
            Ws16 = cload("Ws16_s", [P, IN_FEAT], F16, Ws16_d)
            Wd16 = cload("Wd16_s", [P, IN_FEAT], F16, Wd16_d)
            We16 = cload("We16_s", [P, IN_FEAT], F16, We16_d)
            adiag = cload("adiag_s", [P, HEADS], F16, adiag_d)
            arep = cload("arep_s", [P, IN_FEAT], F16, arep_d)
            ident = cload("ident_s", [P, P], F16, ident_d)
            xTl8 = cload("xTl8_s", [P, N_CHUNKS * P], F8, xTl8_d)
            smap = cload("smap_s", [P, NB * (BATCH_N // 16)], I16, smap_d)

            bias4 = cb.tile([P, 1], FP, name="bias4")
            nc.vector.memset(bias4[:], -EXP_SHIFT)

            Wdp2 = Wdp[:].rearrange("p (two f) -> p two f", two=2)

            # w~[fin,h] = W.T @ (0.6*attn_diag): fp16 for s/e, hi/lo fp8 for d
            wt16 = cb.tile([P, 2 * HEADS], F16, name="wt16")   # [s | e]
            wtd = cb.tile([P, 2 * HEADS], F8, name="wtd")      # [d_hi | d_lo]
            with tc.tile_pool(name="wt_ps", bufs=1, space="PSUM") as wps:
                wp = wps.tile([P, 3 * HEADS], FP)
                nc.tensor.matmul(out=wp[:, 0:8], lhsT=Ws16[:], rhs=adiag[:],
                                 start=True, stop=True)
                nc.tensor.matmul(out=wp[:, 8:16], lhsT=We16[:], rhs=adiag[:],
                                 start=True, stop=True)
                nc.tensor.matmul(out=wp[:, 16:24], lhsT=Wd16[:], rhs=adiag[:],
                                 start=True, stop=True)
                nc.scalar.activation(out=wt16[:], in_=wp[:, 0:16],
                                     func=mybir.ActivationFunctionType.Copy)
                nc.scalar.activation(out=wtd[:, 0:8], in_=wp[:, 16:24],
                                     func=mybir.ActivationFunctionType.Copy)
                nc.vector.tensor_tensor(out=wtd[:, 8:16], in0=wp[:, 16:24],
                                        in1=wtd[:, 0:8],
                                        op=mybir.AluOpType.subtract)
            wtd2 = wtd[:].rearrange("p (two h) -> p two h", two=2)

            # ---------------- phase B
            with (
                tc.tile_pool(name="eb_sb", bufs=4) as eb,
                tc.tile_pool(name="eb_T", bufs=2, space="PSUM") as epT,
                tc.tile_pool(name="eb_U", bufs=2, space="PSUM") as epU,
                tc.tile_pool(name="eb_lin", bufs=2, space="PSUM") as epL,
            ):
                for st in range(n_st):
                    ULst = eb.tile([96, 6 * 136], F16, tag="ULst", bufs=6)

                    for half in range(2):
                        meg = eb.tile([P, HALF_BYTES], U8, tag="meg", bufs=10)
                        nc.sync.dma_start(
                            out=meg[:],
                            in_=mega_d[:, st * ST_BYTES + half * HALF_BYTES:
                                       st * ST_BYTES
                                       + (half + 1) * HALF_BYTES])
                        hb = 0
                        T_ps = epT.tile([P, HALF_E], FP, tag="T")
                        U_ps = epU.tile([96, 3 * 136], FP, tag="U")
                        linT = epL.tile([P, 8 * HEADS], FP, tag="lin")
                        lin_ps = linT[:]

                        # dummy-T absorbs T_ps-free; then the first real
                        # T matmul only waits on the mega DMA
                        nc.tensor.matmul(out=T_ps[:1, 0:1],
                                         lhsT=ident[:, :1], rhs=ident[:, :1],
                                         start=True, stop=True)

                        for tl in range(8):
                            o = hb + tl * TILE_BYTES
                            xs = meg[:, o:o + 256].bitcast(F16)
                            xd = meg[:, o + 256:o + 384].bitcast(F8)
                            ef = meg[:, o + 384:o + 640].bitcast(F16)
                            xd2 = xd.unsqueeze(1).to_broadcast([P, 2, TILE_E])
                            ts = slice(tl * TILE_E, (tl + 1) * TILE_E)
                            nc.tensor.matmul(out=T_ps[:, ts], lhsT=xs,
                                             rhs=WsT[:], start=True,
                                             stop=False)
                            nc.tensor.matmul(out=T_ps[:, ts], lhsT=xd2,
                                             rhs=Wdp2, start=False, stop=False,
                                             perf_mode=DR)
                            nc.tensor.matmul(out=T_ps[:, ts], lhsT=ef,
                                             rhs=WeT[:], start=False,
                                             stop=True)
                            # one start=True marks the whole lin bank
                            # (tl==0); later tiles rely on bank-granular
                            # pending-zero to self-initialize their columns
                            ls = slice(tl * HEADS, (tl + 1) * HEADS)
                            nc.tensor.matmul(out=linT[:, ls], lhsT=xs,
                                             rhs=wt16[:, 0:8],
                                             start=(tl == 0), stop=False,
                                             skip_group_check=True)
                            nc.tensor.matmul(out=linT[:, ls], lhsT=xd2,
                                             rhs=wtd2, start=False,
                                             stop=False, perf_mode=DR,
                                             skip_group_check=True)
                            nc.tensor.matmul(out=linT[:, ls], lhsT=ef,
                                             rhs=wt16[:, 8:16], start=False,
                                             stop=False,
                                             skip_group_check=True)

                        # |T| -> SBUF fp16 (one scalar-engine pass)
                        absT = eb.tile([P, HALF_E], F16, tag="absT")
                        nc.scalar.activation(
                            out=absT[:], in_=T_ps[:],
                            func=mybir.ActivationFunctionType.Abs)
                        # (0.4*attn) * |T| in place (DVE fp16 2x)
                        nc.vector.tensor_tensor(
                            out=absT[:].rearrange("p (t f) -> p t f", t=8),
                            in0=absT[:].rearrange("p (t f) -> p t f", t=8),
                            in1=arep[:].unsqueeze(1).to_broadcast(
                                [P, 8, IN_FEAT]),
                            op=mybir.AluOpType.mult)
                        # pairwise d-tree: Pool s1-s3, DVE s4
                        r1 = eb.tile([P, HALF_E // 2], F16, tag="r1")
                        a0 = absT[:].rearrange("p (g d) -> p g d", d=16)
                        nc.gpsimd.tensor_tensor(
                            out=r1[:].rearrange("p (g d) -> p g d", d=8),
                            in0=a0[:, :, 0:8], in1=a0[:, :, 8:16],
                            op=mybir.AluOpType.add)
                        r2 = eb.tile([P, HALF_E // 4], F16, tag="r2")
                        a1 = r1[:].rearrange("p (g d) -> p g d", d=8)
                        nc.gpsimd.tensor_tensor(
                            out=r2[:].rearrange("p (g d) -> p g d", d=4),
                            in0=a1[:, :, 0:4], in1=a1[:, :, 4:8],
                            op=mybir.AluOpType.add)
                        r3 = eb.tile([P, HALF_E // 8], F16, tag="r3")
                        a2 = r2[:].rearrange("p (g d) -> p g d", d=4)
                        nc.vector.tensor_tensor(
                            out=r3[:].rearrange("p (g d) -> p g d", d=2),
                            in0=a2[:, :, 0:2], in1=a2[:, :, 2:4],
                            op=mybir.AluOpType.add)
                        red = eb.tile([P, HALF_E // 16], F16, tag="red")
                        a3 = r3[:].rearrange("p (g d) -> p g d", d=2)
                        nc.vector.tensor_tensor(
                            out=red[:].rearrange("p (g d) -> p g d", d=1),
                            in0=a3[:, :, 0:1], in1=a3[:, :, 1:2],
                            op=mybir.AluOpType.add)
                        # fold red into lin PSUM: score = lin + red
                        nc.tensor.matmul(out=lin_ps, lhsT=ident[:],
                                         rhs=red[:], start=False, stop=True,
                                         skip_group_check=True)
                        # absorb U_ps-free wait before the scatters
                        nc.tensor.matmul(out=U_ps[:1, :1], lhsT=ident[:, :1],
                                         rhs=ident[:, :1], start=True,
                                         stop=True)

                        # msg: 8 tiles x [128 feat + 8 z cols] fp16
                        msg = eb.tile([P, 8 * 136], F16, tag="msg", bufs=6)
                        mv = msg[:].rearrange("p (t f) -> p t f", t=8)
                        nc.scalar.activation(
                            out=mv[:, :, 128:136],
                            in_=lin_ps.rearrange("p (t h) -> p t h", t=8),
                            func=mybir.ActivationFunctionType.Exp,
                            bias=bias4[:], scale=1.0)
                        # absorb the exp wait for the scatters
                        nc.tensor.matmul(out=U_ps[:1, 1:2],
                                         lhsT=msg[:, 128:129],
                                         rhs=msg[:, 128:129],
                                         start=True, stop=True)
                        # msg = T * ex (DVE, PSUM read, d-broadcast)
                        exb = mv[:, :, 128:136].unsqueeze(3).to_broadcast(
                            [P, 8, HEADS, HEAD_DIM])
                        nc.vector.tensor_tensor(
                            out=mv[:, :, 0:128].rearrange(
                                "p t (h d) -> p t h d", d=HEAD_DIM),
                            in0=T_ps[:].rearrange(
                                "p (t h d) -> p t h d", t=8, d=HEAD_DIM),
                            in1=exb, op=mybir.AluOpType.mult)

                        # scatter: plain fp16 per tile into 3-stacked groups
                        for tl in range(9):
                            sb = hb + 8 * TILE_BYTES + tl * S_BYTES
                            S1 = meg[:, sb:sb + S_BYTES].bitcast(F16)
                            m1 = msg[:, (tl % 8) * 136:(tl % 8) * 136 + 136]
                            g, o3 = tl // 3, tl % 3
                            nc.tensor.matmul(
                                out=U_ps[32 * o3:32 * o3 + 32,
                                         136 * g:136 * g + 136],
                                lhsT=S1, rhs=m1, start=True, stop=True)

                        # evacuate U|z fp16 (one pass per half)
                        nc.scalar.activation(
                            out=ULst[:, half * 408:(half + 1) * 408],
                            in_=U_ps[:],
                            func=mybir.ActivationFunctionType.Copy)

                    nc.scalar.dma_start(
                        out=U_d[st * ST_SLOTS:(st + 1) * ST_SLOTS,
                                0:136].rearrange("(h g p) f -> p h g f",
                                                 h=2, g=3),
                        in_=ULst[:].rearrange("p (h g f) -> p h g f",
                                              h=2, g=3))

            with tc.tile_critical():
                nc.all_engine_barrier()

            # ---------------- phase C
            with (
                tc.tile_pool(name="fin", bufs=3) as fb,
                tc.tile_pool(name="fin_ps", bufs=2, space="PSUM") as fpp,
            ):
                for b in range(NB):
                    nodes = min(BATCH_N, N_LOCAL - b * BATCH_N)
                    kc = (nodes + P - 1) // P
                    nid = kc * P
                    Ug = fb.tile([P, 4 * ROW_E], F16, tag="Ug")
                    nc.gpsimd.dma_gather(
                        out_ap=Ug[:].rearrange("p (c f) -> p c f", c=4)
                        [:, :kc],
                        in_ap=U_d[:],
                        idxs_ap=smap[:, b * 32:b * 32 + nid // 16],
                        num_idxs=nid, num_idxs_reg=nid, elem_size=ROW_E)

                    fps = fpp.tile([P, 4 * IN_FEAT], FP, tag="fps")
                    for j in range(kc):
                        xn = xTl8[:, (b * 4 + j) * P:(b * 4 + j + 1) * P]
                        xn2 = xn.unsqueeze(1).to_broadcast([P, 2, P])
                        nc.tensor.matmul(
                            out=fps[:, j * IN_FEAT:(j + 1) * IN_FEAT],
                            lhsT=xn2, rhs=Wdp2, start=True, stop=True,
                            perf_mode=DR)

                    Ugv = Ug[:].rearrange("p (c f) -> p c f", c=4)
                    zv = Ugv[:, :kc, 128:136]
                    rz = fb.tile([P, 4 * HEADS], FP, tag="rz")
                    rzv = rz[:].rearrange("p (c h) -> p c h", c=4)[:, :kc]
                    nc.vector.tensor_scalar(out=rzv, in0=zv, scalar1=1e-12,
                                            scalar2=None,
                                            op0=mybir.AluOpType.max)
                    nc.vector.reciprocal(out=rzv, in_=rzv)

                    t1 = fb.tile([P, 4 * IN_FEAT], F16, tag="t1")
                    t1v = t1[:].rearrange("p (c h d) -> p c h d", c=4,
                                          d=HEAD_DIM)
                    nc.vector.tensor_tensor(
                        out=t1v[:, :kc],
                        in0=fps[:].rearrange("p (c h d) -> p c h d", c=4,
                                             d=HEAD_DIM)[:, :kc],
                        in1=zv.unsqueeze(3).to_broadcast(
                            [P, kc, HEADS, HEAD_DIM]),
                        op=mybir.AluOpType.mult)
                    t2 = fb.tile([P, 4 * IN_FEAT], F16, tag="t2")
                    t2v = t2[:].rearrange("p (c f) -> p c f", c=4)
                    nc.vector.tensor_tensor(
                        out=t2v[:, :kc], in0=Ugv[:, :kc, 0:128],
                        in1=t1[:].rearrange("p (c f) -> p c f", c=4)[:, :kc],
                        op=mybir.AluOpType.subtract)
                    t3 = fb.tile([P, 4 * IN_FEAT], F16, tag="t3")
                    t3v = t3[:].rearrange("p (c h d) -> p c h d", c=4,
                                          d=HEAD_DIM)
                    nc.gpsimd.tensor_tensor(
                        out=t3v[:, :kc],
                        in0=t2[:].rearrange("p (c h d) -> p c h d", c=4,
                                            d=HEAD_DIM)[:, :kc],
                        in1=rzv.unsqueeze(3).to_broadcast(
                            [P, kc, HEADS, HEAD_DIM]),
                        op=mybir.AluOpType.mult)
                    ob = fb.tile([P, 4 * IN_FEAT], FP, tag="ob")
                    obv = ob[:].rearrange("p (c f) -> p c f", c=4)
                    nc.scalar.activation(
                        out=obv[:, :kc],
                        in_=t3[:].rearrange("p (c f) -> p c f", c=4)[:, :kc],
                        func=mybir.ActivationFunctionType.Relu)
                    lo = b * BATCH_N
                    if nodes == BATCH_N:
                        nc.scalar.dma_start(
                            out=out_d[lo:lo + BATCH_N, :].rearrange(
                                "(c p) f -> p c f", p=P),
                            in_=obv)
                    else:
                        full = nodes // P
                        for j in range(full):
                            nc.scalar.dma_start(
                                out=out_d[lo + j * P:lo + (j + 1) * P, :],
                                in_=obv[:, j])
                        rem = nodes - full * P
                        if rem:
                            nc.scalar.dma_start(
                                out=out_d[lo + full * P:lo + nodes, :],
                                in_=obv[:rem, full])
    nc.compile()
    return nc


_PROGRAM_CACHE = {}


def kernel(**inputs) -> np.ndarray:
    in_maps, T_tiles = _prep_cores(**inputs)
    if T_tiles not in _PROGRAM_CACHE:
        _PROGRAM_CACHE[T_tiles] = build_program(T_tiles)
    nc = _PROGRAM_CACHE[T_tiles]
    res = run_bass_kernel_spmd(nc, in_maps, list(range(N_CORES)))
    out = np.concatenate([np.asarray(res.results[k]["out"])
                          for k in range(N_CORES)], axis=0)
    return out.astype(np.float32)



# revision 7
# speedup vs baseline: 1.3755x; 1.3755x over previous
"""GATv2Conv-with-edge-features Trainium2 kernel (8-core SPMD, edge-sharded by dst).

Self-contained: hardcodes problem shapes (N=50000 nodes, E=800000 edges,
128 feat, 8 heads x 16). Core k owns dst nodes [6250k, 6250(k+1)) and the
edges pointing into them. Edges are sorted by dst and packed into tiles of
<=128 edges covering <=32 consecutive dst nodes; tile windows PARTITION the
local node range so every node (incl. degree-0) has exactly one slot.

Single fused loop over chunks of 8 tiles (1024 edges), software-pipelined
with a 2-chunk skew so every engine streams:
  stage A (chunk c):   one mega-DMA (xs16/ef16/xd8/S16/xdn8 feature-major),
                       T = xs@Ws + xd@Wd + ef@We per tile (fp16 matmuls +
                       one fp8 DoubleRow for the hi/lo-split dst term),
                       lin = (0.6 attn)^T T via tiny matmuls,
                       T16 = Copy(T_ps) on Act, |T| via bitwise-and (DVE 4x),
                       F = |T| * (0.4 attn) (DVE 2x, d-major columns),
  stage B (chunk c-2): score = lin + sum_d F via 16 tiny identity-matmul
                       folds on PE (no vector tree), ex = Exp(score-4) into
                       the z-columns of msg, msg = T16*ex (DVE 4 tiles /
                       Pool 4 tiles), scatter S^T@msg into U|z PSUM (8
                       exclusive regions), fdst per slot from xdn via the
                       same fp8 DR matmul (exact cancellation), then
                       out = relu((U - fdst*z) / max(z,2^-14)) in fp16 and
                       one 512B-descriptor DMA of slot-ordered rows.
Host does layout only: pack/sort/gather into the mega buffer, and scatter
the slot-ordered output rows back to node order (numpy indexing).
"""
import numpy as np
import ml_dtypes

import concourse.bacc as bacc
import concourse.tile as tile
import concourse.mybir as mybir
from concourse.bass_utils import run_bass_kernel_spmd

N_NODES = 50000
N_CORES = 8
N_LOCAL = N_NODES // N_CORES          # 6250
IN_FEAT = 128
HEADS = 8
HEAD_DIM = 16
TILE_E = 128
TILE_W = 32
CH_TILES = 8                          # tiles per chunk
EXP_SHIFT = 4.0
EPS_Z = 2.0 ** -14                    # fp16-safe softmax-denominator floor
P = 128
FP = mybir.dt.float32
F16 = mybir.dt.float16
F8 = mybir.dt.float8e4
U8 = mybir.dt.uint8
NP8 = ml_dtypes.float8_e4m3
NP16 = np.float16

TILE_BYTES = 704                      # xs16 256 | ef16 256 | xd8 128 | S16 64
XDN_BYTES = 3 * P                     # 384: 3 col-groups x (3x32 slots + pad)
CH_BYTES = CH_TILES * TILE_BYTES + XDN_BYTES   # 5888
SKEW = 2

# d-major output-feature permutation: T column j = feature PJ[j]
PJ = np.array([h * HEAD_DIM + d for d in range(HEAD_DIM) for h in range(HEADS)])


# ---------------------------------------------------------------- host prep

def _pack_core(dst_local, n_local):
    """Tiles of <=TILE_E edges whose dst windows PARTITION [0, n_local)."""
    deg = np.bincount(dst_local, minlength=n_local)
    tb = [0]
    cnt = 0
    for n in range(n_local):
        d = int(deg[n])
        assert d <= TILE_E, f"node degree {d} exceeds tile capacity {TILE_E}"
        if (cnt + d > TILE_E or n - tb[-1] >= TILE_W) and n > tb[-1]:
            tb.append(n)
            cnt = 0
        cnt += d
    tb = np.asarray(tb + [n_local])
    t_of_node = np.searchsorted(tb, np.arange(n_local), side="right") - 1
    s_of_node = np.arange(n_local) - tb[t_of_node]
    tile_cnt = np.array([int(deg[tb[t]:tb[t + 1]].sum())
                         for t in range(len(tb) - 1)])
    assert (tile_cnt <= TILE_E).all()
    return tb, tile_cnt, t_of_node, s_of_node


def _prep_cores(x, efeat, src, dst, W_src, b_src, W_dst, b_dst, W_edge, attn):
    x = np.ascontiguousarray(np.asarray(x, np.float32))
    efeat = np.asarray(efeat, np.float32)
    src = np.asarray(src).astype(np.int64)
    dst = np.asarray(dst).astype(np.int64)
    W_src = np.asarray(W_src, np.float32)
    W_dst = np.asarray(W_dst, np.float32)
    W_edge = np.asarray(W_edge, np.float32)
    attn = np.asarray(attn, np.float32)
    assert np.abs(np.asarray(b_src)).max() == 0
    assert np.abs(np.asarray(b_dst)).max() == 0

    x16 = x.astype(NP16)
    x8 = x.astype(NP8)
    ef16 = efeat.astype(NP16)

    per_core = []
    core_T = []
    for k in range(N_CORES):
        lo = k * N_LOCAL
        eidx = np.nonzero((dst >= lo) & (dst < lo + N_LOCAL))[0]
        dl = (dst[eidx] - lo).astype(np.int64)
        order = np.argsort(dl, kind="stable")
        eidx, dl = eidx[order], dl[order]
        per_core.append((eidx, dl) + _pack_core(dl, N_LOCAL))
        core_T.append(len(per_core[-1][2]) - 1)

    T_tiles = max(core_T)
    T_tiles = ((T_tiles + CH_TILES - 1) // CH_TILES) * CH_TILES
    n_ch = T_tiles // CH_TILES

    # weights: output columns permuted to d-major
    WsT16 = np.ascontiguousarray(W_src[PJ].T.astype(NP16))      # [128,128]
    WeT16 = np.ascontiguousarray(W_edge[PJ].T.astype(NP16))
    WdT = W_dst[PJ].T                                           # fp32
    Wd_hi = WdT.astype(NP8)
    Wd_lo = (WdT - Wd_hi.astype(np.float32)).astype(NP8)
    Wd8p = np.ascontiguousarray(np.concatenate([Wd_hi, Wd_lo], axis=1))

    attn_flat = np.zeros((IN_FEAT, HEADS), np.float32)
    for h in range(HEADS):
        attn_flat[h * HEAD_DIM:(h + 1) * HEAD_DIM, h] = attn[h]
    wts16 = np.ascontiguousarray((W_src.T @ (0.6 * attn_flat)).astype(NP16))
    wte16 = np.ascontiguousarray((W_edge.T @ (0.6 * attn_flat)).astype(NP16))
    wtd32 = W_dst.T @ (0.6 * attn_flat)
    wtd_hi = wtd32.astype(NP8)
    wtd_lo = (wtd32 - wtd_hi.astype(np.float32)).astype(NP8)
    wtd8p = np.ascontiguousarray(np.concatenate([wtd_hi, wtd_lo], axis=1))

    arep16 = np.ascontiguousarray(np.broadcast_to(
        (0.4 * attn.T).reshape(1, IN_FEAT), (P, IN_FEAT)).astype(NP16))
    ident16 = np.eye(P, dtype=NP16)

    in_maps = []
    node_maps = []
    for k in range(N_CORES):
        eidx, dl, tb, tcnt, t_of_node, s_of_node = per_core[k]
        nt = len(tb) - 1

        mega = np.zeros((P, n_ch * CH_BYTES), np.uint8)
        pos = 0
        for t in range(nt):
            c, tl = t // CH_TILES, t % CH_TILES
            base = c * CH_BYTES + tl * TILE_BYTES
            cnt = int(tcnt[t])
            if cnt:
                e_ids = eidx[pos:pos + cnt]
                d_loc = dl[pos:pos + cnt]
                pos += cnt
                mega[:, base:base + 256].view(NP16)[:, :cnt] = \
                    x16[src[e_ids]].T
                mega[:, base + 256:base + 512].view(NP16)[:, :cnt] = \
                    ef16[e_ids].T
                mega[:, base + 512:base + 640].view(NP8)[:, :cnt] = \
                    x8[d_loc + k * N_LOCAL].T
                sview = mega[:, base + 640:base + 704].view(NP16)
                sview[np.arange(cnt), d_loc - tb[t]] = NP16(1.0)
            # per-slot node features for the fdst recompute
            w = int(tb[t + 1] - tb[t])
            xb = c * CH_BYTES + CH_TILES * TILE_BYTES \
                + (tl // 3) * P + (tl % 3) * TILE_W
            mega[:, xb:xb + w].view(NP8)[:, :w] = \
                x8[k * N_LOCAL + tb[t]:k * N_LOCAL + tb[t + 1]].T
        assert pos == len(eidx)

        in_maps.append(dict(
            mega_in=mega, WsT16=WsT16, WeT16=WeT16, Wd8p=Wd8p,
            wts16=wts16, wte16=wte16, wtd8p=wtd8p,
            arep16=arep16, ident16=ident16,
        ))
        node_maps.append((t_of_node, s_of_node))
    return in_maps, node_maps, T_tiles


# ------------------------------------------------------------- bass program

def build_program(T_tiles):
    nc = bacc.Bacc("TRN2", target_bir_lowering=False, debug=False,
                   num_devices=N_CORES)
    n_ch = T_tiles // CH_TILES

    mega_d = nc.dram_tensor("mega_in", [P, n_ch * CH_BYTES], U8,
                            kind="ExternalInput")
    WsT_d = nc.dram_tensor("WsT16", [P, IN_FEAT], F16, kind="ExternalInput")
    WeT_d = nc.dram_tensor("WeT16", [P, IN_FEAT], F16, kind="ExternalInput")
    Wd8_d = nc.dram_tensor("Wd8p", [P, 2 * IN_FEAT], F8, kind="ExternalInput")
    wts_d = nc.dram_tensor("wts16", [P, HEADS], F16, kind="ExternalInput")
    wte_d = nc.dram_tensor("wte16", [P, HEADS], F16, kind="ExternalInput")
    wtd_d = nc.dram_tensor("wtd8p", [P, 2 * HEADS], F8, kind="ExternalInput")
    arep_d = nc.dram_tensor("arep16", [P, IN_FEAT], F16, kind="ExternalInput")
    ident_d = nc.dram_tensor("ident16", [P, P], F16, kind="ExternalInput")
    out_d = nc.dram_tensor("out_sl", [96, n_ch * 3 * IN_FEAT], F16,
                           kind="ExternalOutput")

    with tile.TileContext(nc) as tc:
        with tc.tile_pool(name="const", bufs=1) as cb:
            def cload(name, shape, dt, dram):
                t = cb.tile(shape, dt, name=name)
                nc.sync.dma_start(out=t[:], in_=dram[:])
                return t

            WsT = cload("WsT_s", [P, IN_FEAT], F16, WsT_d)
            WeT = cload("WeT_s", [P, IN_FEAT], F16, WeT_d)
            Wd8 = cload("Wd8_s", [P, 2 * IN_FEAT], F8, Wd8_d)
            wts = cload("wts_s", [P, HEADS], F16, wts_d)
            wte = cload("wte_s", [P, HEADS], F16, wte_d)
            wtd = cload("wtd_s", [P, 2 * HEADS], F8, wtd_d)
            arep = cload("arep_s", [P, IN_FEAT], F16, arep_d)
            ident = cload("ident_s", [P, P], F16, ident_d)

            bias4 = cb.tile([P, 1], FP, name="bias4")
            nc.vector.memset(bias4[:], -EXP_SHIFT)

            Wd8s = Wd8[:].rearrange("p (two f) -> p two f", two=2)
            wtds = wtd[:].rearrange("p (two h) -> p two h", two=2)

            with (
                tc.tile_pool(name="meg", bufs=5) as megp,
                tc.tile_pool(name="t16", bufs=3) as t16p,
                tc.tile_pool(name="ff", bufs=3) as ffp,
                tc.tile_pool(name="msg", bufs=3) as msgp,
                tc.tile_pool(name="fin", bufs=3) as finp,
                tc.tile_pool(name="ps_t", bufs=2, space="PSUM") as pst,
                tc.tile_pool(name="ps_ul", bufs=3, space="PSUM") as psul,
                tc.tile_pool(name="ps_fd", bufs=1, space="PSUM") as psfd,
            ):
                megs, t16s, ffs, uls, msgs = {}, {}, {}, {}, {}

                for c in range(n_ch + SKEW):
                    if c < n_ch:
                        meg = megp.tile([P, CH_BYTES], U8, tag="meg")
                        megs[c] = meg
                        nc.sync.dma_start(
                            out=meg[:],
                            in_=mega_d[:, c * CH_BYTES:(c + 1) * CH_BYTES])

                        T_ps = pst.tile([P, CH_TILES * TILE_E], FP, tag="T")
                        UL = psul.tile([P, 3 * 136 + 64], FP, tag="UL")
                        uls[c] = UL
                        # dummies absorb the psum-free waits so the real
                        # matmuls only wait on the mega DMA
                        nc.tensor.matmul(out=T_ps[:1, 0:1],
                                         lhsT=ident[:, :1], rhs=ident[:, :1],
                                         start=True, stop=True)
                        nc.tensor.matmul(out=UL[:1, 408:409],
                                         lhsT=ident[:, :1], rhs=ident[:, :1],
                                         start=True, stop=True,
                                         skip_group_check=True)
                        lin = UL[:, 408:472]

                        for tl in range(CH_TILES):
                            o = tl * TILE_BYTES
                            xs = meg[:, o:o + 256].bitcast(F16)
                            ef = meg[:, o + 256:o + 512].bitcast(F16)
                            xd2 = meg[:, o + 512:o + 640].bitcast(F8) \
                                .unsqueeze(1).to_broadcast([P, 2, TILE_E])
                            ts = slice(tl * TILE_E, (tl + 1) * TILE_E)
                            nc.tensor.matmul(out=T_ps[:, ts], lhsT=xs,
                                             rhs=WsT[:], start=True,
                                             stop=False)
                            nc.tensor.matmul(out=T_ps[:, ts], lhsT=xd2,
                                             rhs=Wd8s, start=False,
                                             stop=False,
                                             perf_mode=mybir.MatmulPerfMode
                                             .DoubleRow)
                            nc.tensor.matmul(out=T_ps[:, ts], lhsT=ef,
                                             rhs=WeT[:], start=False,
                                             stop=True)
                            ls = slice(408 + tl * HEADS,
                                       408 + (tl + 1) * HEADS)
                            nc.tensor.matmul(out=UL[:, ls], lhsT=xs,
                                             rhs=wts[:], start=(tl == 0),
                                             stop=False,
                                             skip_group_check=True)
                            nc.tensor.matmul(out=UL[:, ls], lhsT=xd2,
                                             rhs=wtds, start=False,
                                             stop=False,
                                             perf_mode=mybir.MatmulPerfMode
                                             .DoubleRow,
                                             skip_group_check=True)
                            nc.tensor.matmul(out=UL[:, ls], lhsT=ef,
                                             rhs=wte[:], start=False,
                                             stop=False,
                                             skip_group_check=True)

                        T16 = t16p.tile([P, CH_TILES * TILE_E], F16,
                                        tag="T16")
                        t16s[c] = T16
                        nc.scalar.activation(
                            out=T16[:], in_=T_ps[:],
                            func=mybir.ActivationFunctionType.Copy)
                        FF = ffp.tile([P, CH_TILES * TILE_E], F16, tag="FF")
                        ffs[c] = FF
                        nc.vector.tensor_scalar(
                            out=FF[:].bitcast(mybir.dt.int16),
                            in0=T16[:].bitcast(mybir.dt.int16),
                            scalar1=0x7FFF, scalar2=None,
                            op0=mybir.AluOpType.bitwise_and)
                        nc.vector.tensor_tensor(
                            out=FF[:].rearrange("p (t f) -> p t f", t=8),
                            in0=FF[:].rearrange("p (t f) -> p t f", t=8),
                            in1=arep[:].unsqueeze(1).to_broadcast(
                                [P, CH_TILES, IN_FEAT]),
                            op=mybir.AluOpType.mult)

                    if c >= SKEW:
                        j = c - SKEW
                        meg, T16, FF, UL = megs.pop(j), t16s.pop(j), \
                            ffs.pop(j), uls.pop(j)
                        lin = UL[:, 408:472]
                        Fv = FF[:].rearrange("p (t d h) -> p t d h",
                                             t=8, d=16)
                        for d in range(HEAD_DIM):
                            nc.tensor.matmul(
                                out=lin, lhsT=ident[:], rhs=Fv[:, :, d, :],
                                start=False, stop=(d == HEAD_DIM - 1),
                                skip_group_check=True)

                        msg = msgp.tile([P, CH_TILES * 136], F16, tag="msg")
                        msgs[j] = msg
                        mv = msg[:].rearrange("p (t f) -> p t f", t=8)
                        nc.scalar.activation(
                            out=mv[:, :, 128:136],
                            in_=lin.rearrange("p (t h) -> p t h", t=8),
                            func=mybir.ActivationFunctionType.Exp,
                            bias=bias4[:], scale=1.0)
                        exb = mv[:, :, 128:136].unsqueeze(2).to_broadcast(
                            [P, CH_TILES, HEAD_DIM, HEADS])
                        mfeat = msg[:].rearrange(
                            "p (t f) -> p t f", t=8)[:, :, 0:128].rearrange(
                            "p t (d h) -> p t d h", d=16)
                        t16v = T16[:].rearrange("p (t d h) -> p t d h",
                                                t=8, d=16)
                        nc.vector.tensor_tensor(
                            out=mfeat[:, 0:3], in0=t16v[:, 0:3],
                            in1=exb[:, 0:3], op=mybir.AluOpType.mult)
                        nc.gpsimd.tensor_tensor(
                            out=mfeat[:, 3:8], in0=t16v[:, 3:8],
                            in1=exb[:, 3:8], op=mybir.AluOpType.mult)

                        # scatter into 8 exclusive U regions + fdst recompute
                        fd_ps = psfd.tile([P, 3 * IN_FEAT], FP, tag="fd")
                        nc.tensor.matmul(out=fd_ps[:1, 0:1],
                                         lhsT=ident[:, :1], rhs=ident[:, :1],
                                         start=True, stop=True)
                        for tl in range(CH_TILES):
                            sb = tl * TILE_BYTES + 640
                            S1 = meg[:, sb:sb + 64].bitcast(F16)
                            g, o3 = tl // 3, tl % 3
                            nc.tensor.matmul(
                                out=UL[32 * o3:32 * o3 + 32,
                                       136 * g:136 * g + 136],
                                lhsT=S1, rhs=msg[:, tl * 136:tl * 136 + 136],
                                start=True, stop=True)
                        xdn = meg[:, CH_TILES * TILE_BYTES:
                                  CH_TILES * TILE_BYTES + XDN_BYTES] \
                            .bitcast(F8)
                        for g in range(3):
                            xp = xdn[:, P * g:P * g + P] \
                                .unsqueeze(1).to_broadcast([P, 2, P])
                            nc.tensor.matmul(
                                out=fd_ps[:, 128 * g:128 * g + 128],
                                lhsT=xp, rhs=Wd8s, start=True, stop=True,
                                perf_mode=mybir.MatmulPerfMode.DoubleRow)

                        # finishing math: out = relu((U - fdst*z)/max(z,eps))
                        fin = finp.tile([P, 24 + 24 + 3 * 384], F16,
                                        tag="fin")
                        z16 = fin[:, 0:24]
                        rz16 = fin[:, 24:48]
                        fd16 = fin[:, 48:432]
                        t1v = fin[:, 432:816].rearrange(
                            "p (g d h) -> p g d h", g=3, d=16)
                        o16 = fin[:, 816:1200]
                        nc.scalar.activation(
                            out=z16.rearrange("p (g h) -> p g h", g=3),
                            in_=UL[:, 0:408].rearrange("p (g f) -> p g f",
                                                       g=3)[:, :, 128:136],
                            func=mybir.ActivationFunctionType.Copy)
                        nc.vector.tensor_scalar(
                            out=rz16, in0=z16, scalar1=EPS_Z, scalar2=None,
                            op0=mybir.AluOpType.max)
                        with nc.allow_low_precision(
                                reason="1/z fits fp16; z >= 2^-14"):
                            nc.vector.reciprocal(out=rz16, in_=rz16)
                        nc.scalar.activation(
                            out=fd16, in_=fd_ps[:],
                            func=mybir.ActivationFunctionType.Copy)
                        zb = z16.rearrange("p (g h) -> p g h", g=3) \
                            .unsqueeze(2).to_broadcast([P, 3, 16, HEADS])
                        nc.gpsimd.tensor_tensor(
                            out=t1v,
                            in0=fd16.rearrange("p (g d h) -> p g d h",
                                               g=3, d=16),
                            in1=zb, op=mybir.AluOpType.mult)
                        o16v = o16.rearrange("p (g d h) -> p g d h",
                                             g=3, d=16)
                        nc.vector.tensor_tensor(
                            out=o16v,
                            in0=UL[:, 0:408].rearrange("p (g f) -> p g f",
                                                       g=3)[:, :, 0:128]
                            .rearrange("p g (d h) -> p g d h", d=16),
                            in1=t1v, op=mybir.AluOpType.subtract)
                        rzb = rz16.rearrange("p (g h) -> p g h", g=3) \
                            .unsqueeze(2).to_broadcast([P, 3, 16, HEADS])
                        nc.vector.tensor_tensor(
                            out=o16v, in0=o16v, in1=rzb,
                            op=mybir.AluOpType.mult)
                        nc.vector.tensor_scalar(
                            out=o16, in0=o16, scalar1=0.0, scalar2=None,
                            op0=mybir.AluOpType.max)
                        nc.scalar.dma_start(
                            out=out_d[:, j * 384:(j + 1) * 384],
                            in_=o16[0:96, :])
                        msgs.pop(j)
    nc.compile()
    return nc


_PROGRAM_CACHE = {}


def kernel(**inputs) -> np.ndarray:
    in_maps, node_maps, T_tiles = _prep_cores(**inputs)
    if T_tiles not in _PROGRAM_CACHE:
        _PROGRAM_CACHE[T_tiles] = build_program(T_tiles)
    nc = _PROGRAM_CACHE[T_tiles]
    res = run_bass_kernel_spmd(nc, in_maps, list(range(N_CORES)))

    n_ch = T_tiles // CH_TILES
    inv = np.empty_like(PJ)
    inv[PJ] = np.arange(IN_FEAT)      # vals col j -> feature PJ[j]
    outs = []
    for k in range(N_CORES):
        sl = np.asarray(res.results[k]["out_sl"])      # [96, n_ch*384] f16
        sl = sl.reshape(96, n_ch, 3, IN_FEAT)
        t_of_node, s_of_node = node_maps[k]
        c = t_of_node // CH_TILES
        tl = t_of_node % CH_TILES
        part = 32 * (tl % 3) + s_of_node
        g = tl // 3
        vals = sl[part, c, g, :]                        # [6250, 128]
        outs.append(vals[:, inv].astype(np.float32))
    return np.concatenate(outs, axis=0)


# revision 8
# speedup vs baseline: 1.4611x; 1.0622x over previous
"""GATv2Conv-with-edge-features Trainium2 kernel (8-core SPMD, edge-sharded by dst).

Self-contained: hardcodes problem shapes (N=50000 nodes, E=800000 edges,
128 feat, 8 heads x 16). Core k owns dst nodes [6250k, 6250(k+1)) and the
edges pointing into them. Edges are sorted by dst and packed into tiles of
<=128 edges covering <=32 consecutive dst nodes; tile windows PARTITION the
local node range so every node (incl. degree-0) has exactly one slot.

Single fused loop over chunks of 8 tiles (1024 edges), software-pipelined
with a 2-chunk skew so every engine streams:
  stage A (chunk c):   one mega-DMA (xs16/ef16/xd8/S16/xdn8 feature-major),
                       T = xs@Ws + xd@Wd + ef@We per tile (fp16 matmuls +
                       one fp8 DoubleRow for the hi/lo-split dst term),
                       lin = (0.6 attn)^T T via tiny matmuls,
                       T16 = Copy(T_ps) on Act, |T| via bitwise-and (DVE 4x),
                       F = |T| * (0.4 attn) (DVE 2x, d-major columns),
  stage B (chunk c-2): score = lin + sum_d F via 16 tiny identity-matmul
                       folds on PE (no vector tree), ex = Exp(score-4) into
                       the z-columns of msg, msg = T16*ex (DVE 4 tiles /
                       Pool 4 tiles), scatter S^T@msg into U|z PSUM (8
                       exclusive regions), fdst per slot from xdn via the
                       same fp8 DR matmul (exact cancellation), then
                       out = relu((U - fdst*z) / max(z,2^-14)) in fp16 and
                       one 512B-descriptor DMA of slot-ordered rows.
Host does layout only: pack/sort/gather into the mega buffer, and scatter
the slot-ordered output rows back to node order (numpy indexing).
"""
import numpy as np
import ml_dtypes

import concourse.bacc as bacc
import concourse.tile as tile
import concourse.mybir as mybir
from concourse.bass_utils import run_bass_kernel_spmd

N_NODES = 50000
N_CORES = 8
N_LOCAL = N_NODES // N_CORES          # 6250
IN_FEAT = 128
HEADS = 8
HEAD_DIM = 16
TILE_E = 128
TILE_W = 32
CH_TILES = 8                          # tiles per chunk
EXP_SHIFT = 4.0
EPS_Z = 2.0 ** -14                    # fp16-safe softmax-denominator floor
P = 128
FP = mybir.dt.float32
F16 = mybir.dt.float16
F8 = mybir.dt.float8e4
U8 = mybir.dt.uint8
NP8 = ml_dtypes.float8_e4m3
NP16 = np.float16

TILE_BYTES = 704                      # xs16 256 | ef16 256 | xd8 128 | S16 64
XDN_BYTES = 3 * P                     # 384: 3 col-groups x (3x32 slots + pad)
CH_BYTES = CH_TILES * TILE_BYTES + XDN_BYTES   # 5888
SKEW = 2

# d-major output-feature permutation: T column j = feature PJ[j]
PJ = np.array([h * HEAD_DIM + d for d in range(HEAD_DIM) for h in range(HEADS)])


# ---------------------------------------------------------------- host prep

def _pack_core(dst_local, n_local):
    """Tiles of <=TILE_E edges whose dst windows PARTITION [0, n_local)."""
    deg = np.bincount(dst_local, minlength=n_local)
    tb = [0]
    cnt = 0
    for n in range(n_local):
        d = int(deg[n])
        assert d <= TILE_E, f"node degree {d} exceeds tile capacity {TILE_E}"
        if (cnt + d > TILE_E or n - tb[-1] >= TILE_W) and n > tb[-1]:
            tb.append(n)
            cnt = 0
        cnt += d
    tb = np.asarray(tb + [n_local])
    t_of_node = np.searchsorted(tb, np.arange(n_local), side="right") - 1
    s_of_node = np.arange(n_local) - tb[t_of_node]
    tile_cnt = np.array([int(deg[tb[t]:tb[t + 1]].sum())
                         for t in range(len(tb) - 1)])
    assert (tile_cnt <= TILE_E).all()
    return tb, tile_cnt, t_of_node, s_of_node


def _prep_cores(x, efeat, src, dst, W_src, b_src, W_dst, b_dst, W_edge, attn):
    x = np.ascontiguousarray(np.asarray(x, np.float32))
    efeat = np.asarray(efeat, np.float32)
    src = np.asarray(src).astype(np.int64)
    dst = np.asarray(dst).astype(np.int64)
    W_src = np.asarray(W_src, np.float32)
    W_dst = np.asarray(W_dst, np.float32)
    W_edge = np.asarray(W_edge, np.float32)
    attn = np.asarray(attn, np.float32)
    assert np.abs(np.asarray(b_src)).max() == 0
    assert np.abs(np.asarray(b_dst)).max() == 0

    x16 = x.astype(NP16)
    x8 = x.astype(NP8)
    ef16 = efeat.astype(NP16)

    per_core = []
    core_T = []
    for k in range(N_CORES):
        lo = k * N_LOCAL
        eidx = np.nonzero((dst >= lo) & (dst < lo + N_LOCAL))[0]
        dl = (dst[eidx] - lo).astype(np.int64)
        order = np.argsort(dl, kind="stable")
        eidx, dl = eidx[order], dl[order]
        per_core.append((eidx, dl) + _pack_core(dl, N_LOCAL))
        core_T.append(len(per_core[-1][2]) - 1)

    T_tiles = max(core_T)
    T_tiles = ((T_tiles + CH_TILES - 1) // CH_TILES) * CH_TILES
    n_ch = T_tiles // CH_TILES

    # weights: output columns permuted to d-major
    WsT16 = np.ascontiguousarray(W_src[PJ].T.astype(NP16))      # [128,128]
    WeT16 = np.ascontiguousarray(W_edge[PJ].T.astype(NP16))
    WdT = W_dst[PJ].T                                           # fp32
    Wd_hi = WdT.astype(NP8)
    Wd_lo = (WdT - Wd_hi.astype(np.float32)).astype(NP8)
    Wd8p = np.ascontiguousarray(np.concatenate([Wd_hi, Wd_lo], axis=1))

    attn_flat = np.zeros((IN_FEAT, HEADS), np.float32)
    for h in range(HEADS):
        attn_flat[h * HEAD_DIM:(h + 1) * HEAD_DIM, h] = attn[h]
    wts16 = np.ascontiguousarray((W_src.T @ (0.6 * attn_flat)).astype(NP16))
    wte16 = np.ascontiguousarray((W_edge.T @ (0.6 * attn_flat)).astype(NP16))
    wtd32 = W_dst.T @ (0.6 * attn_flat)
    wtd_hi = wtd32.astype(NP8)
    wtd_lo = (wtd32 - wtd_hi.astype(np.float32)).astype(NP8)
    wtd8p = np.ascontiguousarray(np.concatenate([wtd_hi, wtd_lo], axis=1))

    arep16 = np.ascontiguousarray(np.broadcast_to(
        (0.4 * attn.T).reshape(1, IN_FEAT), (P, IN_FEAT)).astype(NP16))
    ident16 = np.eye(P, dtype=NP16)

    in_maps = []
    node_maps = []
    for k in range(N_CORES):
        eidx, dl, tb, tcnt, t_of_node, s_of_node = per_core[k]
        nt = len(tb) - 1

        mega = np.zeros((P, n_ch * CH_BYTES), np.uint8)
        pos = 0
        for t in range(nt):
            c, tl = t // CH_TILES, t % CH_TILES
            base = c * CH_BYTES + tl * TILE_BYTES
            cnt = int(tcnt[t])
            if cnt:
                e_ids = eidx[pos:pos + cnt]
                d_loc = dl[pos:pos + cnt]
                pos += cnt
                mega[:, base:base + 256].view(NP16)[:, :cnt] = \
                    x16[src[e_ids]].T
                mega[:, base + 256:base + 512].view(NP16)[:, :cnt] = \
                    ef16[e_ids].T
                mega[:, base + 512:base + 640].view(NP8)[:, :cnt] = \
                    x8[d_loc + k * N_LOCAL].T
                sview = mega[:, base + 640:base + 704].view(NP16)
                sview[np.arange(cnt), d_loc - tb[t]] = NP16(1.0)
            # per-slot node features for the fdst recompute
            w = int(tb[t + 1] - tb[t])
            xb = c * CH_BYTES + CH_TILES * TILE_BYTES \
                + (tl // 3) * P + (tl % 3) * TILE_W
            mega[:, xb:xb + w].view(NP8)[:, :w] = \
                x8[k * N_LOCAL + tb[t]:k * N_LOCAL + tb[t + 1]].T
        assert pos == len(eidx)

        in_maps.append(dict(
            mega_in=mega, WsT16=WsT16, WeT16=WeT16, Wd8p=Wd8p,
            wts16=wts16, wte16=wte16, wtd8p=wtd8p,
            arep16=arep16, ident16=ident16,
        ))
        node_maps.append((t_of_node, s_of_node))
    return in_maps, node_maps, T_tiles


# ------------------------------------------------------------- bass program

def build_program(T_tiles):
    nc = bacc.Bacc("TRN2", target_bir_lowering=False, debug=False,
                   num_devices=N_CORES)
    n_ch = T_tiles // CH_TILES

    mega_d = nc.dram_tensor("mega_in", [P, n_ch * CH_BYTES], U8,
                            kind="ExternalInput")
    WsT_d = nc.dram_tensor("WsT16", [P, IN_FEAT], F16, kind="ExternalInput")
    WeT_d = nc.dram_tensor("WeT16", [P, IN_FEAT], F16, kind="ExternalInput")
    Wd8_d = nc.dram_tensor("Wd8p", [P, 2 * IN_FEAT], F8, kind="ExternalInput")
    wts_d = nc.dram_tensor("wts16", [P, HEADS], F16, kind="ExternalInput")
    wte_d = nc.dram_tensor("wte16", [P, HEADS], F16, kind="ExternalInput")
    wtd_d = nc.dram_tensor("wtd8p", [P, 2 * HEADS], F8, kind="ExternalInput")
    arep_d = nc.dram_tensor("arep16", [P, IN_FEAT], F16, kind="ExternalInput")
    ident_d = nc.dram_tensor("ident16", [P, P], F16, kind="ExternalInput")
    out_d = nc.dram_tensor("out_sl", [96, n_ch * 3 * IN_FEAT], F16,
                           kind="ExternalOutput")

    with tile.TileContext(nc) as tc:
        with tc.tile_pool(name="const", bufs=1) as cb:
            def cload(name, shape, dt, dram):
                t = cb.tile(shape, dt, name=name)
                nc.sync.dma_start(out=t[:], in_=dram[:])
                return t

            WsT = cload("WsT_s", [P, IN_FEAT], F16, WsT_d)
            WeT = cload("WeT_s", [P, IN_FEAT], F16, WeT_d)
            Wd8 = cload("Wd8_s", [P, 2 * IN_FEAT], F8, Wd8_d)
            wts = cload("wts_s", [P, HEADS], F16, wts_d)
            wte = cload("wte_s", [P, HEADS], F16, wte_d)
            wtd = cload("wtd_s", [P, 2 * HEADS], F8, wtd_d)
            arep = cload("arep_s", [P, IN_FEAT], F16, arep_d)
            ident = cload("ident_s", [P, P], F16, ident_d)

            bias4 = cb.tile([P, 1], FP, name="bias4")
            nc.vector.memset(bias4[:], -EXP_SHIFT)

            Wd8s = Wd8[:].rearrange("p (two f) -> p two f", two=2)
            wtds = wtd[:].rearrange("p (two h) -> p two h", two=2)

            with (
                tc.tile_pool(name="meg", bufs=5) as megp,
                tc.tile_pool(name="t16", bufs=3) as t16p,
                tc.tile_pool(name="ff", bufs=3) as ffp,
                tc.tile_pool(name="msg", bufs=3) as msgp,
                tc.tile_pool(name="fin", bufs=3) as finp,
                tc.tile_pool(name="fd16", bufs=3) as fd16p,
                tc.tile_pool(name="ps_t", bufs=2, space="PSUM") as pst,
                tc.tile_pool(name="ps_ul", bufs=3, space="PSUM") as psul,
                tc.tile_pool(name="ps_fd", bufs=1, space="PSUM") as psfd,
            ):
                megs, t16s, ffs, uls, fd16s = {}, {}, {}, {}, {}

                for c in range(n_ch + SKEW):
                    j = c - SKEW          # stage-B chunk
                    i = c - 1             # fdst-precompute chunk

                    # ---- mega prefetch
                    if c < n_ch:
                        meg = megp.tile([P, CH_BYTES], U8, tag="meg")
                        megs[c] = meg
                        nc.sync.dma_start(
                            out=meg[:],
                            in_=mega_d[:, c * CH_BYTES:(c + 1) * CH_BYTES])

                    # ---- B-front: folds, exp, msg (chunk j)
                    if j >= 0:
                        T16j, FFj, ULj = t16s.pop(j), ffs.pop(j), uls.pop(j)
                        megj = megs.pop(j)
                        linj = ULj[:, 408:472]
                        Fv = FFj[:].rearrange("p (t d h) -> p t d h",
                                              t=8, d=16)
                        for d in range(HEAD_DIM):
                            nc.tensor.matmul(
                                out=linj, lhsT=ident[:], rhs=Fv[:, :, d, :],
                                start=False, stop=(d == HEAD_DIM - 1),
                                skip_group_check=True)
                        msg = msgp.tile([P, CH_TILES * 136], F16, tag="msg")
                        mv = msg[:].rearrange("p (t f) -> p t f", t=8)
                        nc.scalar.activation(
                            out=mv[:, :, 128:136],
                            in_=linj.rearrange("p (t h) -> p t h", t=8),
                            func=mybir.ActivationFunctionType.Exp,
                            bias=bias4[:], scale=1.0)
                        exb = mv[:, :, 128:136].unsqueeze(2).to_broadcast(
                            [P, CH_TILES, HEAD_DIM, HEADS])
                        mfeat = msg[:].rearrange(
                            "p (t f) -> p t f", t=8)[:, :, 0:128].rearrange(
                            "p t (d h) -> p t d h", d=16)
                        t16v = T16j[:].rearrange("p (t d h) -> p t d h",
                                                 t=8, d=16)
                        nc.vector.tensor_tensor(
                            out=mfeat[:, 0:4], in0=t16v[:, 0:4],
                            in1=exb[:, 0:4], op=mybir.AluOpType.mult)
                        nc.gpsimd.tensor_tensor(
                            out=mfeat[:, 4:8], in0=t16v[:, 4:8],
                            in1=exb[:, 4:8], op=mybir.AluOpType.mult)

                    # ---- A-compute: T + lin matmuls (chunk c)
                    if c < n_ch:
                        T_ps = pst.tile([P, CH_TILES * TILE_E], FP, tag="T")
                        UL = psul.tile([P, 3 * 136 + 64], FP, tag="UL")
                        uls[c] = UL
                        # dummies absorb the psum-free waits so the real
                        # matmuls only wait on the mega DMA
                        nc.tensor.matmul(out=T_ps[:1, 0:1],
                                         lhsT=ident[:, :1], rhs=ident[:, :1],
                                         start=True, stop=True)
                        nc.tensor.matmul(out=UL[:1, 408:409],
                                         lhsT=ident[:, :1], rhs=ident[:, :1],
                                         start=True, stop=True,
                                         skip_group_check=True)
                        for tl in range(CH_TILES):
                            o = tl * TILE_BYTES
                            xs = meg[:, o:o + 256].bitcast(F16)
                            ef = meg[:, o + 256:o + 512].bitcast(F16)
                            xd2 = meg[:, o + 512:o + 640].bitcast(F8) \
                                .unsqueeze(1).to_broadcast([P, 2, TILE_E])
                            ts = slice(tl * TILE_E, (tl + 1) * TILE_E)
                            nc.tensor.matmul(out=T_ps[:, ts], lhsT=xs,
                                             rhs=WsT[:], start=True,
                                             stop=False)
                            nc.tensor.matmul(out=T_ps[:, ts], lhsT=xd2,
                                             rhs=Wd8s, start=False,
                                             stop=False,
                                             perf_mode=mybir.MatmulPerfMode
                                             .DoubleRow)
                            nc.tensor.matmul(out=T_ps[:, ts], lhsT=ef,
                                             rhs=WeT[:], start=False,
                                             stop=True)
                            ls = slice(408 + tl * HEADS,
                                       408 + (tl + 1) * HEADS)
                            nc.tensor.matmul(out=UL[:, ls], lhsT=xs,
                                             rhs=wts[:], start=(tl == 0),
                                             stop=False,
                                             skip_group_check=True)
                            nc.tensor.matmul(out=UL[:, ls], lhsT=xd2,
                                             rhs=wtds, start=False,
                                             stop=False,
                                             perf_mode=mybir.MatmulPerfMode
                                             .DoubleRow,
                                             skip_group_check=True)
                            nc.tensor.matmul(out=UL[:, ls], lhsT=ef,
                                             rhs=wte[:], start=False,
                                             stop=False,
                                             skip_group_check=True)

                    # ---- B-mid: scatters (chunk j)
                    if j >= 0:
                        for tl in range(CH_TILES):
                            sb = tl * TILE_BYTES + 640
                            S1 = megj[:, sb:sb + 64].bitcast(F16)
                            g, o3 = tl // 3, tl % 3
                            nc.tensor.matmul(
                                out=ULj[32 * o3:32 * o3 + 32,
                                        136 * g:136 * g + 136],
                                lhsT=S1, rhs=msg[:, tl * 136:tl * 136 + 136],
                                start=True, stop=True)

                    # ---- A-evac: T16, |T|, F (chunk c)
                    if c < n_ch:
                        T16 = t16p.tile([P, CH_TILES * TILE_E], F16,
                                        tag="T16")
                        t16s[c] = T16
                        nc.scalar.activation(
                            out=T16[:], in_=T_ps[:],
                            func=mybir.ActivationFunctionType.Copy)
                        FF = ffp.tile([P, CH_TILES * TILE_E], F16, tag="FF")
                        ffs[c] = FF
                        nc.vector.tensor_scalar(
                            out=FF[:].bitcast(mybir.dt.int16),
                            in0=T16[:].bitcast(mybir.dt.int16),
                            scalar1=0x7FFF, scalar2=None,
                            op0=mybir.AluOpType.bitwise_and)
                        nc.vector.tensor_tensor(
                            out=FF[:].rearrange("p (t f) -> p t f", t=8),
                            in0=FF[:].rearrange("p (t f) -> p t f", t=8),
                            in1=arep[:].unsqueeze(1).to_broadcast(
                                [P, CH_TILES, IN_FEAT]),
                            op=mybir.AluOpType.mult)

                    # ---- B-tail: out = relu((U - fdst*z)/max(z,eps))
                    if j >= 0:
                        fd16j = fd16s.pop(j)
                        fin = finp.tile([P, 24 + 24 + 2 * 384], F16,
                                        tag="fin")
                        z16 = fin[:, 0:24]
                        rz16 = fin[:, 24:48]
                        t1v = fin[:, 48:432].rearrange(
                            "p (g d h) -> p g d h", g=3, d=16)
                        o16 = fin[:, 432:816]
                        nc.scalar.activation(
                            out=z16.rearrange("p (g h) -> p g h", g=3),
                            in_=ULj[:, 0:408].rearrange("p (g f) -> p g f",
                                                        g=3)[:, :, 128:136],
                            func=mybir.ActivationFunctionType.Copy)
                        nc.vector.tensor_scalar(
                            out=rz16, in0=z16, scalar1=EPS_Z, scalar2=None,
                            op0=mybir.AluOpType.max)
                        with nc.allow_low_precision(
                                reason="1/z fits fp16; z >= 2^-14"):
                            nc.vector.reciprocal(out=rz16, in_=rz16)
                        zb = z16.rearrange("p (g h) -> p g h", g=3) \
                            .unsqueeze(2).to_broadcast([P, 3, 16, HEADS])
                        nc.gpsimd.tensor_tensor(
                            out=t1v,
                            in0=fd16j[:].rearrange("p (g d h) -> p g d h",
                                                   g=3, d=16),
                            in1=zb, op=mybir.AluOpType.mult)
                        o16v = o16.rearrange("p (g d h) -> p g d h",
                                             g=3, d=16)
                        nc.vector.tensor_tensor(
                            out=o16v,
                            in0=ULj[:, 0:408].rearrange("p (g f) -> p g f",
                                                        g=3)[:, :, 0:128]
                            .rearrange("p g (d h) -> p g d h", d=16),
                            in1=t1v, op=mybir.AluOpType.subtract)
                        rzb = rz16.rearrange("p (g h) -> p g h", g=3) \
                            .unsqueeze(2).to_broadcast([P, 3, 16, HEADS])
                        nc.vector.tensor_tensor(
                            out=o16v, in0=o16v, in1=rzb,
                            op=mybir.AluOpType.mult)
                        nc.vector.tensor_scalar(
                            out=o16, in0=o16, scalar1=0.0, scalar2=None,
                            op0=mybir.AluOpType.max)
                        nc.scalar.dma_start(
                            out=out_d[:, j * 384:(j + 1) * 384],
                            in_=o16[0:96, :])

                    # ---- fdst precompute (chunk i, one iter ahead of use)
                    if 0 <= i < n_ch:
                        megi = megs[i]
                        fd_ps = psfd.tile([P, 3 * IN_FEAT], FP, tag="fd")
                        nc.tensor.matmul(out=fd_ps[:1, 0:1],
                                         lhsT=ident[:, :1], rhs=ident[:, :1],
                                         start=True, stop=True)
                        xdn = megi[:, CH_TILES * TILE_BYTES:
                                   CH_TILES * TILE_BYTES + XDN_BYTES] \
                            .bitcast(F8)
                        for g in range(3):
                            xp = xdn[:, P * g:P * g + P] \
                                .unsqueeze(1).to_broadcast([P, 2, P])
                            nc.tensor.matmul(
                                out=fd_ps[:, 128 * g:128 * g + 128],
                                lhsT=xp, rhs=Wd8s, start=True, stop=True,
                                perf_mode=mybir.MatmulPerfMode.DoubleRow)
                        fd16 = fd16p.tile([P, 3 * IN_FEAT], F16, tag="fd16")
                        fd16s[i] = fd16
                        nc.scalar.activation(
                            out=fd16[:], in_=fd_ps[:],
                            func=mybir.ActivationFunctionType.Copy)
    nc.compile()
    return nc


_PROGRAM_CACHE = {}


def kernel(**inputs) -> np.ndarray:
    in_maps, node_maps, T_tiles = _prep_cores(**inputs)
    if T_tiles not in _PROGRAM_CACHE:
        _PROGRAM_CACHE[T_tiles] = build_program(T_tiles)
    nc = _PROGRAM_CACHE[T_tiles]
    res = run_bass_kernel_spmd(nc, in_maps, list(range(N_CORES)))

    n_ch = T_tiles // CH_TILES
    inv = np.empty_like(PJ)
    inv[PJ] = np.arange(IN_FEAT)      # vals col j -> feature PJ[j]
    outs = []
    for k in range(N_CORES):
        sl = np.asarray(res.results[k]["out_sl"])      # [96, n_ch*384] f16
        sl = sl.reshape(96, n_ch, 3, IN_FEAT)
        t_of_node, s_of_node = node_maps[k]
        c = t_of_node // CH_TILES
        tl = t_of_node % CH_TILES
        part = 32 * (tl % 3) + s_of_node
        g = tl // 3
        vals = sl[part, c, g, :]                        # [6250, 128]
        outs.append(vals[:, inv].astype(np.float32))
    return np.concatenate(outs, axis=0)
